# revision 38
# baseline (speedup 1.0000x reference)
"""GroupGRUCell with shared schema-pool parameters — Trainium2 Bass kernel.

Problem shapes (hardcoded): B=256 batch, U=64 GRU units, DIN=H=256, S=8 schemas.
  Wx[u] = sum_s sw_x[u,s] * pool_x[s].T   (per-unit weights from shared pool)
  gate_x = x @ Wx ; gate_h = h @ Wh ; standard GRU cell gate math.

Sharding: unit-parallel across 8 NeuronCores (8 units per core); the schema
pool is replicated per core.

The schema combine runs on the PE: with the pool laid out
[(s,dhi):128, (dlo, pool, o)] (d = dhi*16+dlo, x/h pools interleaved per dlo)
and block-diagonal mixing matrices M[(s,dhi),(u,dhi')] = sw[u,s]*d(dhi,dhi'),
  M.T @ slab-chunk = W[(u,dhi), (dlo, pool, o)]
computes all 16 per-unit weight matrices in 64 384-col matmuls (384 keeps
each chunk pool-pure and within one PSUM bank). PSUM results are cast to
bf16 (alternating ACT/DVE), then one strided SBUF->SBUF DMA per (u,kc)
unflattens [8 stride-8 part, 16*1536] -> [128 part, 1536] (3KB descriptor
lines), yielding the [d, (pool,o)] layout the per-unit matmuls need.

Scheduling notes baked in below:
- All scatters ride the sync HWDGE ring: a dma_start ahead of sigmoid in
  the scalar sequencer FIFO stalls gate math on descriptor-ring drain.
- DMA dst APs must keep a plain contiguous partition dim (transposed or
  split-partition dst APs lower to wrong data).
- Slab pieces + all input DMAs are issued up front, slab first (HBM
  streams in priority order); each HWDGE dma_start costs ~700ns of
  sequencer issue time, so fewer+fatter DMAs win.
- Gate math runs in bf16: DVE t1/t2/e/o, ACT sigmoid/tanh (table set
  pinned by a warm-up call), GPSIMD the subtract.
"""

import numpy as np
import ml_dtypes

B, U, DIN, H, S = 256, 64, 256, 256, 8
NCORES = 8
UC = U // NCORES  # units per core
O3 = 3 * H        # 768
KC = DIN // 128   # 2 contraction chunks
MC = B // 128     # 2 batch chunks
FSL = 16 * 2 * O3  # slab free size 24576 (dlo, pool, o)
CW = 384           # combine chunk width (pool-pure: 1536 = 4*384; 1 PSUM bank)
NCH = FSL // CW    # 64 combine chunks

BF16 = ml_dtypes.bfloat16


def _build_program():
    from contextlib import ExitStack

    import concourse.bacc as bacc
    import concourse.mybir as mybir
    import concourse.tile as tile

    bf = mybir.dt.bfloat16
    f32 = mybir.dt.float32
    AF = mybir.ActivationFunctionType
    ALU = mybir.AluOpType

    nc = bacc.Bacc("TRN2", target_bir_lowering=False, debug=False)

    xT = nc.dram_tensor("xT", [128, UC, KC, B], bf, kind="ExternalInput")
    hT = nc.dram_tensor("hT", [128, UC, KC, B], bf, kind="ExternalInput")
    hbB = nc.dram_tensor("hbB", [128, UC, MC, H], bf, kind="ExternalInput")
    slab = nc.dram_tensor("slab", [128, FSL], bf, kind="ExternalInput")
    Mxh = nc.dram_tensor("Mxh", [128, 2, 128], bf, kind="ExternalInput")
    hy = nc.dram_tensor("hy", [128, UC, MC * H], bf, kind="ExternalOutput")

    with tile.TileContext(nc) as tc, ExitStack() as ctx:
        pconst = ctx.enter_context(tc.tile_pool(name="pconst", bufs=1))
        pslab = ctx.enter_context(tc.tile_pool(name="pslab", bufs=1))
        pgtmp = ctx.enter_context(tc.tile_pool(name="pgtmp", bufs=6))
        ppsum_c = ctx.enter_context(tc.tile_pool(name="ppsc", bufs=4, space="PSUM"))
        ppsum = ctx.enter_context(tc.tile_pool(name="ppsum", bufs=2, space="PSUM"))

        # --- constant/staging tiles ---
        m_all = pconst.tile([128, 2, 128], bf, tag="mxh")
        nc.sync.dma_start(out=m_all, in_=Mxh[:, :, :])
        mx_sb = m_all[:, 0, :]
        mh_sb = m_all[:, 1, :]

        # Pin the ACT table set to one containing sigmoid+tanh+copy up front,
        # so no mid-kernel ACT_TABLE_LOAD stalls the first real sigmoid.
        warm = pconst.tile([128, 2], f32, tag="warm")
        nc.scalar.activation(out=warm, in_=mx_sb[:, 0:2], func=AF.Sigmoid)
        warm2 = pconst.tile([128, 2], f32, tag="warm2")
        nc.scalar.activation(out=warm2, in_=mx_sb[:, 0:2], func=AF.Tanh)

        NQ = 8  # slab streamed in pieces, all DMAs issued up front
        QW = FSL // NQ
        CPQ = NCH // NQ  # combine chunks per slab piece

        # slab first in the sync ring so it streams at full HBM rate, then
        # the (later-needed) x/h/hidden inputs, then the scatters.
        slq = []
        for q in range(NQ):
            sq = pslab.tile([128, QW], bf, tag=f"slq{q}")
            nc.sync.dma_start(out=sq, in_=slab[:, q * QW : (q + 1) * QW])
            slq.append(sq)
        xt_all = pconst.tile([128, UC, KC, B], bf, tag="xta")
        nc.sync.dma_start(out=xt_all, in_=xT[:, :, :, :])
        ht_all = pconst.tile([128, UC, KC, B], bf, tag="hta")
        nc.sync.dma_start(out=ht_all, in_=hT[:, :, :, :])
        hb_all = pconst.tile([128, UC, MC, H], bf, tag="hba")
        nc.sync.dma_start(out=hb_all, in_=hbB[:, :, :, :])

        cast_sb = pconst.tile([128, FSL], bf, tag="cast")
        wC = []
        for u in range(UC):
            wu = pconst.tile([128, KC, 2, O3], bf, tag=f"w{u}")
            wC.append(wu)

        # --- phase 1: schema combine on the PE ---
        # d = dhi*16 + dlo; combine output partition is dhi-major
        # (p = dhi*8 + u), so each (u,kc) scatter reads 8 stride-8 partitions.
        # All scatters ride the sync ring so the scalar sequencer stays free
        # for sigmoid/tanh (a HWDGE dma_start ahead of them in the scalar
        # FIFO would stall gate math on ring drain).
        cast_r = cast_sb.rearrange("(a b) f -> a b f", b=UC)
        for q in range(NQ):
            sq = slq[q]
            for cc in range(CPQ):
                t = q * CPQ + cc
                m_sb = mh_sb if (t % 4 >= 2) else mx_sb
                ps = ppsum_c.tile([128, CW], f32, tag="cmb")
                nc.tensor.matmul(
                    ps, m_sb, sq[:, cc * CW : (cc + 1) * CW], start=True, stop=True
                )
                dst = cast_sb[:, t * CW : (t + 1) * CW]
                if t % 2 == 0:
                    nc.scalar.activation(out=dst, in_=ps, func=AF.Copy)
                else:
                    nc.vector.tensor_copy(out=dst, in_=ps)
        for u in range(UC):
            for kc in range(KC):
                src = cast_r[kc * 8 : (kc + 1) * 8, u, :]
                # the last unit's scatters ride the (idle) scalar ring:
                # parallel issue + transfer, small enough not to stall the
                # scalar sequencer on ring space ahead of the sigmoids.
                eng = nc.scalar if u >= UC - 2 else nc.sync
                eng.dma_start(out=wC[u][:, kc, :, :], in_=src)

        # --- phase 2: per-unit matmuls + GRU gate math ---
        for u in range(UC):
            w = wC[u]
            if u % 2 == 0:
                o_t = pconst.tile([128, 2, MC, H], bf, tag=f"o{u // 2}")
            for mc in range(MC):
                p_ri = ppsum.tile([128, 512], f32, tag="ri")
                p_nn = ppsum.tile([128, 512], f32, tag="nn")  # [i_n | h_n]
                bs = slice(mc * 128, (mc + 1) * 128)
                for kc in range(KC):
                    lx = xt_all[:, u, kc, bs]
                    nc.tensor.matmul(
                        p_ri, lx, w[:, kc, 0, 0:512], start=(kc == 0), stop=False
                    )
                    nc.tensor.matmul(
                        p_nn[:, 0:256], lx, w[:, kc, 0, 512:O3],
                        start=(kc == 0), stop=(kc == 1),
                    )
                for kc in range(KC):
                    lh = ht_all[:, u, kc, bs]
                    nc.tensor.matmul(
                        p_ri, lh, w[:, kc, 1, 0:512], start=False, stop=(kc == 1)
                    )
                    nc.tensor.matmul(
                        p_nn[:, 256:512], lh, w[:, kc, 1, 512:O3],
                        start=(kc == 0), stop=(kc == 1),
                    )

                # p_ri = [i_r + h_r | i_i + h_i]; sig = sigmoid(p_ri)  (bf16)
                sig = pgtmp.tile([128, 512], bf, tag="sig")
                nc.scalar.activation(out=sig, in_=p_ri, func=AF.Sigmoid)
                t1 = pgtmp.tile([128, H], bf, tag="t1")
                nc.vector.tensor_tensor(
                    out=t1, in0=sig[:, 0:H], in1=p_nn[:, 256:512], op=ALU.mult
                )
                t2 = pgtmp.tile([128, H], bf, tag="t2")
                nc.vector.tensor_tensor(out=t2, in0=t1, in1=p_nn[:, 0:256], op=ALU.add)
                ng = pgtmp.tile([128, H], bf, tag="ng")
                nc.scalar.activation(out=ng, in_=t2, func=AF.Tanh)
                d = pgtmp.tile([128, H], bf, tag="d")
                # DVE for the final unit: shorter critical chain on the tail
                deng = nc.vector if u == UC - 1 else nc.gpsimd
                deng.tensor_tensor(
                    out=d, in0=hb_all[:, u, mc, :], in1=ng, op=ALU.subtract
                )
                e = pgtmp.tile([128, H], bf, tag="e")
                nc.vector.tensor_tensor(out=e, in0=sig[:, H:512], in1=d, op=ALU.mult)
                nc.vector.tensor_tensor(
                    out=o_t[:, u % 2, mc, :], in0=ng, in1=e, op=ALU.add
                )
            if u % 2 == 1:
                if u == UC - 1:
                    # split the final pair so u6's rows ship while u7's
                    # gate math still runs, shortening the output tail
                    nc.scalar.dma_start(out=hy[:, u - 1 : u, :], in_=o_t[:, 0:1])
                    nc.scalar.dma_start(out=hy[:, u : u + 1, :], in_=o_t[:, 1:2])
                else:
                    nc.scalar.dma_start(
                        out=hy[:, u - 1 : u + 1, :],
                        in_=o_t,
                    )

    nc.compile()
    return nc


def _prep_inputs(x, hidden, pool_x, pool_h, sw_x, sw_h):
    """Host-side (free) slicing / transposition / casting per core."""
    # slab[(s,dhi), (dlo, pool, o)] = pool_{pool}[s, o, dhi*16+dlo].T
    pxT = pool_x.transpose(0, 2, 1).reshape(S, 16, 16, O3)
    phT = pool_h.transpose(0, 2, 1).reshape(S, 16, 16, O3)
    slab_h = np.ascontiguousarray(
        np.stack([pxT, phT], axis=3).reshape(128, FSL).astype(BF16)
    )

    def prep_M(sw, us):
        # M[(s,dhi), (dhi'*8 + u)] = sw[us][u, s] * delta(dhi, dhi')
        swc = sw[us].astype(np.float32)  # [UC, S]
        M4 = np.zeros((S, 16, 16, UC), np.float32)
        for i in range(16):
            M4[:, i, i, :] = swc.T
        return np.ascontiguousarray(M4.reshape(128, 128).astype(BF16))

    in_maps = []
    for c in range(NCORES):
        us = slice(c * UC, (c + 1) * UC)
        xc = x[:, us, :]  # [B, UC, DIN]
        hc = hidden[:, us, :]
        # xT[p, u, kc, b] = x[b, u, kc*128+p]
        xT_h = np.ascontiguousarray(
            xc.transpose(2, 1, 0).reshape(KC, 128, UC, B).transpose(1, 2, 0, 3).astype(BF16)
        )
        hT_h = np.ascontiguousarray(
            hc.transpose(2, 1, 0).reshape(KC, 128, UC, B).transpose(1, 2, 0, 3).astype(BF16)
        )
        # hbB[p, u, mc, o] = hidden[mc*128+p, c*UC+u, o]
        hbB_h = np.ascontiguousarray(
            hc.reshape(MC, 128, UC, H).transpose(1, 2, 0, 3).astype(BF16)
        )
        in_maps.append(
            {
                "xT": xT_h,
                "hT": hT_h,
                "hbB": hbB_h,
                "slab": slab_h,
                "Mxh": np.ascontiguousarray(
                    np.stack([prep_M(sw_x, us), prep_M(sw_h, us)], axis=1)
                ),
            }
        )
    return in_maps


_CACHED_NC = None


def _get_nc():
    global _CACHED_NC
    if _CACHED_NC is None:
        _CACHED_NC = _build_program()
    return _CACHED_NC


def kernel(x, hidden, pool_x, pool_h, sw_x, sw_h, _trace=False, _results_holder=None):
    from concourse.bass_utils import run_bass_kernel_spmd

    x = np.asarray(x)
    hidden = np.asarray(hidden)
    pool_x = np.asarray(pool_x)
    pool_h = np.asarray(pool_h)
    sw_x = np.asarray(sw_x)
    sw_h = np.asarray(sw_h)

    nc = _get_nc()
    in_maps = _prep_inputs(x, hidden, pool_x, pool_h, sw_x, sw_h)
    res = run_bass_kernel_spmd(
        nc, in_maps, core_ids=list(range(NCORES)), trace=_trace
    )
    if _results_holder is not None:
        _results_holder.append(res)

    out = np.empty((B, U, H), dtype=np.float32)
    for c in range(NCORES):
        hy_c = np.asarray(res.results[c]["hy"])  # [128, UC, MC*H] bf16
        # [p, u, mc, o] -> [mc*128+p, u, o]
        out[:, c * UC : (c + 1) * UC, :] = (
            hy_c.reshape(128, UC, MC, H)
            .transpose(2, 0, 1, 3)
            .reshape(B, UC, H)
            .astype(np.float32)
        )
    return out


# revision 39
# speedup vs baseline: 1.1520x; 1.1520x over previous
"""GroupGRUCell with shared schema-pool parameters — Trainium2 Bass kernel.

Problem shapes (hardcoded): B=256 batch, U=64 GRU units, DIN=H=256, S=8 schemas.
  Wx[u] = sum_s sw_x[u,s] * pool_x[s].T   (per-unit weights from shared pool)
  gate_x = x @ Wx ; gate_h = h @ Wh ; standard GRU cell gate math.

Sharding: unit-parallel across 8 NeuronCores (8 units per core); the schema
pool is replicated per core.

The schema combine runs on the PE: with the pool laid out
[(s,dhi):128, (dlo, pool, o)] (d = dhi*16+dlo, x/h pools interleaved per dlo)
and block-diagonal mixing matrices M[(s,dhi),(u,dhi')] = sw[u,s]*d(dhi,dhi'),
  M.T @ slab-chunk = W[(u,dhi), (dlo, pool, o)]
computes all 16 per-unit weight matrices in 64 384-col matmuls (384 keeps
each chunk pool-pure and within one PSUM bank). PSUM results are cast to
bf16 (alternating ACT/DVE), then one strided SBUF->SBUF DMA per (u,kc)
unflattens [8 stride-8 part, 16*1536] -> [128 part, 1536] (3KB descriptor
lines), yielding the [d, (pool,o)] layout the per-unit matmuls need.

Scheduling notes baked in below:
- All scatters ride the sync HWDGE ring: a dma_start ahead of sigmoid in
  the scalar sequencer FIFO stalls gate math on descriptor-ring drain.
- DMA dst APs must keep a plain contiguous partition dim (transposed or
  split-partition dst APs lower to wrong data).
- Slab pieces + all input DMAs are issued up front, slab first (HBM
  streams in priority order); each HWDGE dma_start costs ~700ns of
  sequencer issue time, so fewer+fatter DMAs win.
- Gate math runs in bf16: DVE t1/t2/e/o, ACT sigmoid/tanh (table set
  pinned by a warm-up call), GPSIMD the subtract.
"""

import numpy as np
import ml_dtypes

B, U, DIN, H, S = 256, 64, 256, 256, 8
NCORES = 8
UC = U // NCORES  # units per core
O3 = 3 * H        # 768
KC = DIN // 128   # 2 contraction chunks
MC = B // 128     # 2 batch chunks
FSL = 16 * 2 * O3  # slab free size 24576 (dlo, pool, o)
CW = 384           # combine chunk width (pool-pure: 1536 = 4*384; 1 PSUM bank)
NCH = FSL // CW    # 64 combine chunks

BF16 = ml_dtypes.bfloat16


def _build_program():
    from contextlib import ExitStack

    import concourse.bacc as bacc
    import concourse.mybir as mybir
    import concourse.tile as tile

    bf = mybir.dt.bfloat16
    f32 = mybir.dt.float32
    AF = mybir.ActivationFunctionType
    ALU = mybir.AluOpType

    nc = bacc.Bacc("TRN2", target_bir_lowering=False, debug=False)

    xT = nc.dram_tensor("xT", [128, UC, KC, B], bf, kind="ExternalInput")
    hT = nc.dram_tensor("hT", [128, UC, KC, B], bf, kind="ExternalInput")
    hbB = nc.dram_tensor("hbB", [128, UC, MC, H], bf, kind="ExternalInput")
    slab = nc.dram_tensor("slab", [128, FSL], bf, kind="ExternalInput")
    Mxh = nc.dram_tensor("Mxh", [128, 2, 128], bf, kind="ExternalInput")
    hy = nc.dram_tensor("hy", [128, UC, MC * H], bf, kind="ExternalOutput")

    with tile.TileContext(nc) as tc, ExitStack() as ctx:
        pconst = ctx.enter_context(tc.tile_pool(name="pconst", bufs=1))
        pslab = ctx.enter_context(tc.tile_pool(name="pslab", bufs=1))
        pgtmp = ctx.enter_context(tc.tile_pool(name="pgtmp", bufs=6))
        ppsum_c = ctx.enter_context(tc.tile_pool(name="ppsc", bufs=4, space="PSUM"))
        ppsum = ctx.enter_context(tc.tile_pool(name="ppsum", bufs=2, space="PSUM"))

        # --- constant/staging tiles ---
        m_all = pconst.tile([128, 2, 128], bf, tag="mxh")
        nc.sync.dma_start(out=m_all, in_=Mxh[:, :, :])
        mx_sb = m_all[:, 0, :]
        mh_sb = m_all[:, 1, :]

        # Pin the ACT table set to one containing sigmoid+tanh+copy up front,
        # so no mid-kernel ACT_TABLE_LOAD stalls the first real sigmoid.
        warm = pconst.tile([128, 2], f32, tag="warm")
        nc.scalar.activation(out=warm, in_=mx_sb[:, 0:2], func=AF.Sigmoid)
        warm2 = pconst.tile([128, 2], f32, tag="warm2")
        nc.scalar.activation(out=warm2, in_=mx_sb[:, 0:2], func=AF.Tanh)

        NQ = 8  # slab streamed in pieces, all DMAs issued up front
        QW = FSL // NQ
        CPQ = NCH // NQ  # combine chunks per slab piece

        # slab first in the sync ring so it streams at full HBM rate, then
        # the (later-needed) x/h/hidden inputs, then the scatters.
        slq = []
        for q in range(NQ):
            sq = pslab.tile([128, QW], bf, tag=f"slq{q}")
            nc.sync.dma_start(out=sq, in_=slab[:, q * QW : (q + 1) * QW])
            slq.append(sq)
        xt_all = pconst.tile([128, UC, KC, B], bf, tag="xta")
        nc.sync.dma_start(out=xt_all, in_=xT[:, :, :, :])
        ht_all = pconst.tile([128, UC, KC, B], bf, tag="hta")
        nc.sync.dma_start(out=ht_all, in_=hT[:, :, :, :])
        hb_all = pconst.tile([128, UC, MC, H], bf, tag="hba")
        nc.sync.dma_start(out=hb_all, in_=hbB[:, :, :, :])

        cast_sb = pconst.tile([128, FSL], bf, tag="cast")
        wC = []
        for u in range(UC):
            wu = pconst.tile([128, KC, 2, O3], bf, tag=f"w{u}")
            wC.append(wu)

        # --- phase 1: schema combine on the PE ---
        # d = dhi*16 + dlo; combine output partition is dhi-major
        # (p = dhi*8 + u), so each (u,kc) scatter reads 8 stride-8 partitions.
        # All scatters ride the sync ring so the scalar sequencer stays free
        # for sigmoid/tanh (a HWDGE dma_start ahead of them in the scalar
        # FIFO would stall gate math on ring drain).
        cast_r = cast_sb.rearrange("(a b) f -> a b f", b=UC)
        for q in range(NQ):
            sq = slq[q]
            for cc in range(CPQ):
                t = q * CPQ + cc
                m_sb = mh_sb if (t % 4 >= 2) else mx_sb
                ps = ppsum_c.tile([128, CW], f32, tag="cmb")
                nc.tensor.matmul(
                    ps, m_sb, sq[:, cc * CW : (cc + 1) * CW], start=True, stop=True
                )
                dst = cast_sb[:, t * CW : (t + 1) * CW]
                if t % 2 == 0:
                    nc.scalar.activation(out=dst, in_=ps, func=AF.Copy)
                else:
                    nc.vector.tensor_copy(out=dst, in_=ps)
        for u in range(UC):
            for kc in range(KC):
                src = cast_r[kc * 8 : (kc + 1) * 8, u, :]
                # the last unit's scatters ride the (idle) scalar ring:
                # parallel issue + transfer, small enough not to stall the
                # scalar sequencer on ring space ahead of the sigmoids.
                eng = nc.scalar if u == UC - 1 else nc.sync
                eng.dma_start(out=wC[u][:, kc, :, :], in_=src)

        # --- phase 2: per-unit matmuls + GRU gate math ---
        for u in range(UC):
            w = wC[u]
            if u % 2 == 0:
                o_t = pconst.tile([128, 2, MC, H], bf, tag=f"o{u // 2}")
            for mc in range(MC):
                p_ri = ppsum.tile([128, 512], f32, tag="ri")
                p_nn = ppsum.tile([128, 512], f32, tag="nn")  # [i_n | h_n]
                bs = slice(mc * 128, (mc + 1) * 128)
                for kc in range(KC):
                    lx = xt_all[:, u, kc, bs]
                    nc.tensor.matmul(
                        p_ri, lx, w[:, kc, 0, 0:512], start=(kc == 0), stop=False
                    )
                    nc.tensor.matmul(
                        p_nn[:, 0:256], lx, w[:, kc, 0, 512:O3],
                        start=(kc == 0), stop=(kc == 1),
                    )
                for kc in range(KC):
                    lh = ht_all[:, u, kc, bs]
                    nc.tensor.matmul(
                        p_ri, lh, w[:, kc, 1, 0:512], start=False, stop=(kc == 1)
                    )
                    nc.tensor.matmul(
                        p_nn[:, 256:512], lh, w[:, kc, 1, 512:O3],
                        start=(kc == 0), stop=(kc == 1),
                    )

                # p_ri = [i_r + h_r | i_i + h_i]; sig = sigmoid(p_ri)  (bf16)
                sig = pgtmp.tile([128, 512], bf, tag="sig")
                nc.scalar.activation(out=sig, in_=p_ri, func=AF.Sigmoid)
                t1 = pgtmp.tile([128, H], bf, tag="t1")
                nc.vector.tensor_tensor(
                    out=t1, in0=sig[:, 0:H], in1=p_nn[:, 256:512], op=ALU.mult
                )
                t2 = pgtmp.tile([128, H], bf, tag="t2")
                nc.vector.tensor_tensor(out=t2, in0=t1, in1=p_nn[:, 0:256], op=ALU.add)
                ng = pgtmp.tile([128, H], bf, tag="ng")
                nc.scalar.activation(out=ng, in_=t2, func=AF.Tanh)
                d = pgtmp.tile([128, H], bf, tag="d")
                # DVE for the final unit: shorter critical chain on the tail
                deng = nc.vector if u == UC - 1 else nc.gpsimd
                deng.tensor_tensor(
                    out=d, in0=hb_all[:, u, mc, :], in1=ng, op=ALU.subtract
                )
                e = pgtmp.tile([128, H], bf, tag="e")
                nc.vector.tensor_tensor(out=e, in0=sig[:, H:512], in1=d, op=ALU.mult)
                nc.vector.tensor_tensor(
                    out=o_t[:, u % 2, mc, :], in0=ng, in1=e, op=ALU.add
                )
            if u % 2 == 1:
                if u == UC - 1:
                    # split the final pair so u6's rows ship while u7's
                    # gate math still runs, shortening the output tail
                    nc.scalar.dma_start(out=hy[:, u - 1 : u, :], in_=o_t[:, 0:1])
                    nc.scalar.dma_start(out=hy[:, u : u + 1, :], in_=o_t[:, 1:2])
                else:
                    nc.scalar.dma_start(
                        out=hy[:, u - 1 : u + 1, :],
                        in_=o_t,
                    )

    nc.compile()
    return nc


def _prep_inputs(x, hidden, pool_x, pool_h, sw_x, sw_h):
    """Host-side (free) slicing / transposition / casting per core."""
    # slab[(s,dhi), (dlo, pool, o)] = pool_{pool}[s, o, dhi*16+dlo].T
    pxT = pool_x.transpose(0, 2, 1).reshape(S, 16, 16, O3)
    phT = pool_h.transpose(0, 2, 1).reshape(S, 16, 16, O3)
    slab_h = np.ascontiguousarray(
        np.stack([pxT, phT], axis=3).reshape(128, FSL).astype(BF16)
    )

    def prep_M(sw, us):
        # M[(s,dhi), (dhi'*8 + u)] = sw[us][u, s] * delta(dhi, dhi')
        swc = sw[us].astype(np.float32)  # [UC, S]
        M4 = np.zeros((S, 16, 16, UC), np.float32)
        for i in range(16):
            M4[:, i, i, :] = swc.T
        return np.ascontiguousarray(M4.reshape(128, 128).astype(BF16))

    in_maps = []
    for c in range(NCORES):
        us = slice(c * UC, (c + 1) * UC)
        xc = x[:, us, :]  # [B, UC, DIN]
        hc = hidden[:, us, :]
        # xT[p, u, kc, b] = x[b, u, kc*128+p]
        xT_h = np.ascontiguousarray(
            xc.transpose(2, 1, 0).reshape(KC, 128, UC, B).transpose(1, 2, 0, 3).astype(BF16)
        )
        hT_h = np.ascontiguousarray(
            hc.transpose(2, 1, 0).reshape(KC, 128, UC, B).transpose(1, 2, 0, 3).astype(BF16)
        )
        # hbB[p, u, mc, o] = hidden[mc*128+p, c*UC+u, o]
        hbB_h = np.ascontiguousarray(
            hc.reshape(MC, 128, UC, H).transpose(1, 2, 0, 3).astype(BF16)
        )
        in_maps.append(
            {
                "xT": xT_h,
                "hT": hT_h,
                "hbB": hbB_h,
                "slab": slab_h,
                "Mxh": np.ascontiguousarray(
                    np.stack([prep_M(sw_x, us), prep_M(sw_h, us)], axis=1)
                ),
            }
        )
    return in_maps


_CACHED_NC = None


def _get_nc():
    global _CACHED_NC
    if _CACHED_NC is None:
        _CACHED_NC = _build_program()
    return _CACHED_NC


def kernel(x, hidden, pool_x, pool_h, sw_x, sw_h, _trace=False, _results_holder=None):
    from concourse.bass_utils import run_bass_kernel_spmd

    x = np.asarray(x)
    hidden = np.asarray(hidden)
    pool_x = np.asarray(pool_x)
    pool_h = np.asarray(pool_h)
    sw_x = np.asarray(sw_x)
    sw_h = np.asarray(sw_h)

    nc = _get_nc()
    in_maps = _prep_inputs(x, hidden, pool_x, pool_h, sw_x, sw_h)
    res = run_bass_kernel_spmd(
        nc, in_maps, core_ids=list(range(NCORES)), trace=_trace
    )
    if _results_holder is not None:
        _results_holder.append(res)

    out = np.empty((B, U, H), dtype=np.float32)
    for c in range(NCORES):
        hy_c = np.asarray(res.results[c]["hy"])  # [128, UC, MC*H] bf16
        # [p, u, mc, o] -> [mc*128+p, u, o]
        out[:, c * UC : (c + 1) * UC, :] = (
            hy_c.reshape(128, UC, MC, H)
            .transpose(2, 0, 1, 3)
            .reshape(B, UC, H)
            .astype(np.float32)
        )
    return out
